# revision 1
# baseline (speedup 1.0000x reference)
"""Multi-head self-attention Trainium2 kernel (8-core SPMD, full IO).

Problem: x:(2,2048,1024) f32; Wq/Wk/Wv/Wo:(1024,1024); bo:(1024,)
  out = softmax((xWq)(xWk)^T / 8) (xWv) reshaped @ Wo + bo

Sharding: data parallel on batch N=2 x tensor parallel on 16 heads in
4 groups of 4 heads.  Core c handles batch c//4, heads [4*(c%4), 4*(c%4)+4).
Each core computes a partial fc_out product (2048,1024); the host sums the
4 head-group partials per batch and adds the bias.

On-chip layout (per core):
  xT   (1024,2048)  x[n]^T, embed on partitions (8 chunks of 128)
  Q^T/K^T stored as [128, 2, 2048] (dims-chunk on partitions, tokens free)
  V    stored as [128(tokens), 16, 4, 65]; col 64 = ones (denominator trick)
  scores are computed TRANSPOSED: S^T[k,q] so that exp runs on ACT and the
  softmax denominator falls out of the ones-column of V during the O^T
  accumulation (row 64 of the [65,512] psum).  No max subtraction: scores
  are ~N(0,1), bounded well inside fp32 exp range (as in the reference,
  which subtracts max only for stability, not value).
"""

import os

import numpy as np

import concourse.bass as bass
import concourse.tile as tile
from concourse import bacc, mybir
from concourse import bass_utils

F32 = mybir.dt.float32

EMBED = 1024
SEQ = 2048
NB = 2  # batch
HEADS = 16
D = 64  # head dim
NCORES = 8
GROUPS = 4  # head groups (tensor parallel)
HG = HEADS // GROUPS  # heads per core = 4
DG = HG * D  # dims per core = 256

# matmul operand dtype:
#   float32  - exact, 1/4 PE rate
#   float32r - tf32-class (~3e-4 rel), ~2 cyc/row (4-byte stream bound)
#   bfloat16 - ~5e-3 rel, full PE rate, half DMA/SBUF footprint
_MM_DTYPE_NAME = os.environ.get("MHA_MM_DTYPE", "bfloat16")
MM_DTYPE = getattr(mybir.dt, _MM_DTYPE_NAME)
BF16 = mybir.dt.bfloat16

# set by run_cores(); test.py reads exec_time_ns from here
LAST_RESULTS = None
_CACHED_NC = {}


MD = MM_DTYPE  # dtype of matmul-feeding tiles
# DRAM dtype of the big inputs: bf16 inputs are converted host-side (DMA
# cannot cast); f32r shares fp32 bits so DRAM stays f32 + bitcast at DMA.
IN_DT = BF16 if MM_DTYPE == BF16 else F32
IN_NP = None  # numpy dtype for host conversion, set below


def _in_cast(ap):
    """DRAM-side view of an input AP in the matmul dtype."""
    return ap if MD in (F32, BF16) else ap.bitcast(MD)


def build_nc():
    nc = bacc.Bacc("TRN2", target_bir_lowering=False, debug=False,
                   num_devices=NCORES)

    xT = nc.dram_tensor("xT", (EMBED, SEQ), IN_DT, kind="ExternalInput").ap()
    wq = nc.dram_tensor("wq", (EMBED, DG), IN_DT, kind="ExternalInput").ap()
    wk = nc.dram_tensor("wk", (EMBED, DG), IN_DT, kind="ExternalInput").ap()
    wv = nc.dram_tensor("wv", (EMBED, DG), IN_DT, kind="ExternalInput").ap()
    wo = nc.dram_tensor("wo", (DG, EMBED), IN_DT, kind="ExternalInput").ap()
    y = nc.dram_tensor("y", (SEQ, EMBED), F32, kind="ExternalOutput").ap()
    # DRAM bounce buffers for the softmax denominators: SBUF sources can't be
    # partition-broadcast by DMA, DRAM sources can.
    den_dram = nc.dram_tensor("den_scratch", (HG, SEQ), F32).ap()
    rden_dram = nc.dram_tensor("rden_scratch", (HG, SEQ), F32).ap()

    KC = EMBED // 128  # 8 contraction chunks for projections

    with tile.TileContext(nc) as tc:
        with (
            tc.tile_pool(name="weights", bufs=1) as wpool,
            tc.tile_pool(name="qk", bufs=1) as qkpool,
            tc.tile_pool(name="vpool", bufs=1) as vpool,
            tc.tile_pool(name="otpool", bufs=1) as otpool,
            tc.tile_pool(name="xchunk", bufs=2) as xpool,
            tc.tile_pool(name="epool", bufs=12) as epool,
            tc.tile_pool(name="stage", bufs=4) as stpool,
            tc.tile_pool(name="den", bufs=1) as denpool,
            tc.tile_pool(name="rbc", bufs=2) as rbcpool,
            tc.tile_pool(name="ystage", bufs=3) as ypool,
            tc.tile_pool(name="psum", bufs=2, space="PSUM") as pspool,
            tc.tile_pool(name="psum_o", bufs=2, space="PSUM") as popool,
        ):
            # ---- load weights ----
            wq_sb = wpool.tile([128, KC, DG], MD)
            wk_sb = wpool.tile([128, KC, DG], MD)
            wv_sb = wpool.tile([128, KC, DG], MD)
            wo_sb = wpool.tile([128, DG // 128, EMBED], MD)
            nc.sync.dma_start(out=wq_sb, in_=_in_cast(wq).rearrange("(c p) n -> p c n", p=128))
            nc.sync.dma_start(out=wk_sb, in_=_in_cast(wk).rearrange("(c p) n -> p c n", p=128))
            nc.sync.dma_start(out=wv_sb, in_=_in_cast(wv).rearrange("(c p) n -> p c n", p=128))
            nc.sync.dma_start(out=wo_sb, in_=_in_cast(wo).rearrange("(c p) n -> p c n", p=128))

            # per-512-token-chunk tiles: finer dependency granularity lets
            # phase-2 attention start as soon as its chunks are projected
            QTs = [qkpool.tile([128, 2, 512], MD, name=f"qt{t}", tag=f"qt{t}")
                   for t in range(4)]
            KTs = [qkpool.tile([128, 2, 512], MD, name=f"kt{t}", tag=f"kt{t}")
                   for t in range(4)]
            Vs = [vpool.tile([128, 4, HG, D + 1], MD, name=f"v{t}", tag=f"v{t}")
                  for t in range(4)]
            for t in range(4):
                ones_col = Vs[t][:, :, :, D:D + 1]
                nc.vector.memset(
                    ones_col.bitcast(F32) if MD == mybir.dt.float32r
                    else ones_col, 1.0)

            xTr = _in_cast(xT).rearrange("(c p) s -> p c s", p=128)

            # ---- phase 1: projections, one 256-token chunk at a time ----
            TCH = 512
            for tcb in range(SEQ // TCH):
                xc = xpool.tile([128, KC, TCH], MD)
                nc.sync.dma_start(out=xc, in_=xTr[:, :, tcb * TCH:(tcb + 1) * TCH])

                # Q^T and K^T chunks: [dims 128, tokens TCH]
                for wsb, dst in ((wq_sb, QTs), (wk_sb, KTs)):
                    for mt in range(2):
                        ps = popool.tile([128, 512], F32, name="ps1", tag="po")
                        for kc in range(KC):
                            nc.tensor.matmul(
                                ps[:, 0:TCH],
                                wsb[:, kc, mt * 128:(mt + 1) * 128],
                                xc[:, kc, :],
                                start=(kc == 0),
                                stop=(kc == KC - 1),
                            )
                        nc.vector.tensor_copy(
                            out=dst[tcb][:, mt, :], in_=ps[:, 0:TCH])

                # V chunks: [tokens 128, dims 256]
                for ti in range(TCH // 128):
                    tt = tcb * (TCH // 128) + ti
                    ps = popool.tile([128, 512], F32, name="ps1", tag="po")
                    for kc in range(KC):
                        nc.tensor.matmul(
                            ps[:, 0:DG],
                            xc[:, kc, ti * 128:(ti + 1) * 128],
                            wv_sb[:, kc, :],
                            start=(kc == 0),
                            stop=(kc == KC - 1),
                        )
                    nc.vector.tensor_copy(
                        out=Vs[tcb][:, ti, :, 0:D],
                        in_=ps[:, 0:DG].rearrange("p (h d) -> p h d", h=HG))

            # ---- phase 2: attention (scores transposed, head pairs) ----
            # OT2[p, hm, q]: partition p = 64*j + d for head h = 2*hm + j.
            # This matches wo_sb's row layout so fc_out contracts K=128/pair.
            OT2 = otpool.tile([128, 2, SEQ], MD)
            QC = 1024  # q-chunk: one [128, QC] psum = 2 banks, one exp inst

            for hm in range(2):
                for qc in range(SEQ // QC):
                    qs = slice(qc * QC, (qc + 1) * QC)
                    po = [popool.tile([D + 1, QC], F32, name="po", tag="po")
                          for _ in range(2)]
                    for m in range(SEQ // 128):
                        es = []
                        for j in range(2):  # paired heads -> concurrent MMs
                            ps = pspool.tile([128, QC], F32)
                            for ha in range(QC // 512):
                                nc.tensor.matmul(
                                    ps[:, ha * 512:(ha + 1) * 512],
                                    KTs[m // 4][j * D:(j + 1) * D, hm,
                                                (m % 4) * 128:
                                                (m % 4 + 1) * 128],
                                    QTs[2 * qc + ha][j * D:(j + 1) * D, hm, :],
                                    start=True,
                                    stop=True,
                                )
                            e = epool.tile([128, QC], MD)
                            nc.scalar.activation(
                                out=e, in_=ps,
                                func=mybir.ActivationFunctionType.Exp,
                                scale=1.0 / np.sqrt(D),
                            )
                            es.append(e)
                        for j in range(2):
                            for ha in range(QC // 512):
                                nc.tensor.matmul(
                                    po[j][:, ha * 512:(ha + 1) * 512],
                                    Vs[m // 4][:, m % 4, 2 * hm + j, :],
                                    es[j][:, ha * 512:(ha + 1) * 512],
                                    start=(m == 0),
                                    stop=(m == SEQ // 128 - 1),
                                )
                    for j in range(2):
                        h = 2 * hm + j
                        st = stpool.tile([D + 1, QC], F32)
                        nc.vector.tensor_copy(out=st, in_=po[j])
                        ot_dst = OT2[j * D:(j + 1) * D, hm, qs]
                        if MD == BF16:
                            nc.gpsimd.dma_start(out=ot_dst, in_=st[0:D, :])
                        elif MD == F32:
                            nc.sync.dma_start(out=ot_dst, in_=st[0:D, :])
                        else:
                            nc.sync.dma_start(
                                out=ot_dst, in_=st[0:D, :].bitcast(MD))
                        nc.sync.dma_start(
                            out=den_dram[h:h + 1, qs], in_=st[D:D + 1, :])

            # reciprocal, reshaped to use all 128 partitions (free dim 64)
            rsm = denpool.tile([128, HG * SEQ // 128], F32)
            den_r = den_dram.rearrange("h (a b) -> (h a) b", a=32)
            rden_r = rden_dram.rearrange("h (a b) -> (h a) b", a=32)
            nc.sync.dma_start(out=rsm, in_=den_r)
            nc.vector.reciprocal(out=rsm, in_=rsm)
            nc.sync.dma_start(out=rden_r, in_=rsm)

            # normalize O^T rows by 1/denominator (broadcast across partitions)
            for hm in range(2):
                rb = rbcpool.tile([128, SEQ], F32)
                for j in range(2):
                    nc.sync.dma_start(
                        out=rb[j * D:(j + 1) * D, :],
                        in_=rden_dram[2 * hm + j:2 * hm + j + 1, :]
                        .to_broadcast((D, SEQ)))
                nc.vector.tensor_mul(OT2[:, hm, :], OT2[:, hm, :], rb)

            # ---- phase 3: partial fc_out  y = sum_h O_h @ Wo_h (K=128/pair) ----
            for tt in range(SEQ // 128):
                for nch in range(EMBED // 512):
                    ps = pspool.tile([128, QC], F32)
                    for hm in range(2):
                        nc.tensor.matmul(
                            ps[:, 0:512],
                            OT2[:, hm, tt * 128:(tt + 1) * 128],
                            wo_sb[:, hm, nch * 512:(nch + 1) * 512],
                            start=(hm == 0),
                            stop=(hm == 1),
                        )
                    ys = ypool.tile([128, 512], F32)
                    nc.vector.tensor_copy(out=ys, in_=ps[:, 0:512])
                    nc.sync.dma_start(
                        out=y[tt * 128:(tt + 1) * 128, nch * 512:(nch + 1) * 512],
                        in_=ys)

    nc.compile()
    return nc


def shard_inputs(x, Wv, Wk, Wq, Wo):
    """Build the 8 per-core input maps."""
    in_maps = []
    for c in range(NCORES):
        n, g = divmod(c, GROUPS)
        cols = slice(g * DG, (g + 1) * DG)
        wire = np.float32
        if MM_DTYPE == BF16:
            import ml_dtypes
            wire = ml_dtypes.bfloat16
        in_maps.append({
            "xT": np.ascontiguousarray(np.asarray(x[n], np.float32).T).astype(wire),
            "wq": np.ascontiguousarray(np.asarray(Wq, np.float32)[:, cols]).astype(wire),
            "wk": np.ascontiguousarray(np.asarray(Wk, np.float32)[:, cols]).astype(wire),
            "wv": np.ascontiguousarray(np.asarray(Wv, np.float32)[:, cols]).astype(wire),
            "wo": np.ascontiguousarray(np.asarray(Wo, np.float32)[cols, :]).astype(wire),
        })
    return in_maps


def kernel(x, Wv, Wk, Wq, Wo, bo):
    global LAST_RESULTS
    x = np.asarray(x, np.float32)
    in_maps = shard_inputs(x, Wv, Wk, Wq, Wo)

    if "nc" not in _CACHED_NC:
        _CACHED_NC["nc"] = build_nc()
    nc = _CACHED_NC["nc"]

    trace = os.environ.get("MHA_TRACE", "0") == "1"
    res = bass_utils.run_bass_kernel_spmd(
        nc, in_maps, core_ids=list(range(NCORES)), trace=trace)
    LAST_RESULTS = res

    bo = np.asarray(bo, np.float32)
    out = np.empty((NB, SEQ, EMBED), np.float32)
    for n in range(NB):
        acc = res.results[n * GROUPS]["y"].astype(np.float32).copy()
        for g in range(1, GROUPS):
            acc += res.results[n * GROUPS + g]["y"]
        out[n] = acc + bo[None, :]
    return out



# revision 3
# speedup vs baseline: 1.1443x; 1.1443x over previous
"""Multi-head self-attention Trainium2 kernel (8-core SPMD, full IO).

Problem: x:(2,2048,1024) f32; Wq/Wk/Wv/Wo:(1024,1024); bo:(1024,)
  out = softmax((xWq)(xWk)^T / 8) (xWv) reshaped @ Wo + bo

Sharding: data parallel on batch N=2 x tensor parallel on 16 heads in
4 groups of 4 heads.  Core c handles batch c//4, heads [4*(c%4), 4*(c%4)+4).
Each core computes a partial fc_out product (2048,1024) in bf16; the host
sums the 4 head-group partials per batch (f32) and adds the bias.

v2 schedule (single fused pipeline, ACT-engine exp is the critical path):
  - K/V projections first, then Q for the first two 512-token q-chunks.
  - qc-outer attention: for each 512-token q-chunk and head pair hm,
    a software-pipelined m-loop computes scores S^T (PE), exp (ACT, one
    [128,1024] instr covering both heads of the pair), and lagged A@V
    accumulation (PE).  Remaining Q projections and fc_out matmuls for
    completed chunks are fed one-per-iteration into the PE slack.
  - softmax denominator comes from a ones-column of V; normalization is
    done on-chip: gpsimd partition_broadcast + reciprocal + fused
    multiply into the bf16 O^T staging tile (no DRAM round trip).
  - fc_out psum is cast to bf16 and DMA'd out per 128-token block.
"""

import os

import numpy as np

import concourse.bass as bass
import concourse.tile as tile
from concourse import bacc, mybir
from concourse import bass_utils

F32 = mybir.dt.float32
BF16 = mybir.dt.bfloat16

EMBED = 1024
SEQ = 2048
NB = 2  # batch
HEADS = 16
D = 64  # head dim
NCORES = 8
GROUPS = 4  # head groups (tensor parallel)
HG = HEADS // GROUPS  # heads per core = 4
DG = HG * D  # dims per core = 256
KC = EMBED // 128  # 8 contraction chunks for projections
TCH = 512  # token chunk (q-chunk and projection granularity)
NT = SEQ // TCH  # 4 chunks

_MM_DTYPE_NAME = "bfloat16"
MD = BF16

# set by kernel(); test.py reads exec_time_ns from here
LAST_RESULTS = None
_CACHED_NC = {}


def build_nc():
    nc = bacc.Bacc("TRN2", target_bir_lowering=False, debug=False,
                   num_devices=NCORES)

    xT = nc.dram_tensor("xT", (EMBED, SEQ), MD, kind="ExternalInput").ap()
    wq = nc.dram_tensor("wq", (EMBED, DG), MD, kind="ExternalInput").ap()
    wk = nc.dram_tensor("wk", (EMBED, DG), MD, kind="ExternalInput").ap()
    wv = nc.dram_tensor("wv", (EMBED, DG), MD, kind="ExternalInput").ap()
    wo = nc.dram_tensor("wo", (DG, EMBED), MD, kind="ExternalInput").ap()
    y = nc.dram_tensor("y", (SEQ, EMBED), MD, kind="ExternalOutput").ap()

    with tile.TileContext(nc) as tc:
        with (
            tc.tile_pool(name="weights", bufs=1) as wpool,
            tc.tile_pool(name="qk", bufs=1) as qkpool,
            tc.tile_pool(name="vpool", bufs=1) as vpool,
            tc.tile_pool(name="otpool", bufs=1) as otpool,
            tc.tile_pool(name="xchunk", bufs=4) as xpool,
            tc.tile_pool(name="epool", bufs=3) as epool,
            tc.tile_pool(name="scratch", bufs=1) as spool,
            tc.tile_pool(name="stage", bufs=1) as stpool,
            tc.tile_pool(name="rbc", bufs=1) as rbpool,
            tc.tile_pool(name="den", bufs=1) as denpool,
            tc.tile_pool(name="ystage", bufs=2) as ypool,
            tc.tile_pool(name="psum", bufs=2, space="PSUM") as pspool,
            tc.tile_pool(name="psum_o", bufs=1, space="PSUM") as popool,
            tc.tile_pool(name="psum_fcq", bufs=1, space="PSUM") as fqpool,
        ):
            # ---- weights + x chunks (sync queue; K/V first for fast start) --
            wk_sb = wpool.tile([128, KC, DG], MD)
            wv_sb = wpool.tile([128, KC, DG], MD)
            wq_sb = wpool.tile([128, KC, DG], MD)
            wo_sb = wpool.tile([128, DG // 128, EMBED], MD)
            nc.sync.dma_start(out=wk_sb, in_=wk.rearrange("(c p) n -> p c n", p=128))
            nc.sync.dma_start(out=wv_sb, in_=wv.rearrange("(c p) n -> p c n", p=128))

            xTr = xT.rearrange("(c p) s -> p c s", p=128)
            xcs = []
            for t in range(NT):
                xc = xpool.tile([128, KC, TCH], MD, name=f"xc{t}", tag=f"xc{t}")
                nc.sync.dma_start(out=xc, in_=xTr[:, :, t * TCH:(t + 1) * TCH])
                xcs.append(xc)
                if t == 0:
                    nc.sync.dma_start(
                        out=wq_sb, in_=wq.rearrange("(c p) n -> p c n", p=128))
            nc.sync.dma_start(out=wo_sb, in_=wo.rearrange("(c p) n -> p c n", p=128))

            QTs = [qkpool.tile([128, 2, TCH], MD, name=f"qt{t}", tag=f"qt{t}")
                   for t in range(NT)]
            KTs = [qkpool.tile([128, 2, TCH], MD, name=f"kt{t}", tag=f"kt{t}")
                   for t in range(NT)]
            Vs = [vpool.tile([128, 4, HG, D + 1], MD, name=f"v{t}", tag=f"v{t}")
                  for t in range(NT)]
            for t in range(NT):
                nc.vector.memset(Vs[t][:, :, :, D:D + 1], 1.0)

            # ---- K and V projections (interleaved psum accumulation chains) --
            for t in range(NT):
                xc = xcs[t]
                # K^T chunk: two 128-dim chains (mt) in one [128,1024] psum
                pk = pspool.tile([128, 2 * TCH], F32, name="ps", tag="ps")
                for kc in range(KC):
                    for mt in range(2):
                        nc.tensor.matmul(
                            pk[:, mt * TCH:(mt + 1) * TCH],
                            wk_sb[:, kc, mt * 128:(mt + 1) * 128],
                            xc[:, kc, :],
                            start=(kc == 0), stop=(kc == KC - 1))
                nc.vector.tensor_copy(
                    out=KTs[t], in_=pk.rearrange("p (m s) -> p m s", m=2))
                # V chunk: four 128-token blocks, paired into two psum tiles
                for tp in range(2):
                    pv = pspool.tile([128, 2 * TCH], F32, name="ps", tag="ps")
                    for kc in range(KC):
                        for k in range(2):
                            ti = 2 * tp + k
                            nc.tensor.matmul(
                                pv[:, k * TCH:k * TCH + DG],
                                xc[:, kc, ti * 128:(ti + 1) * 128],
                                wv_sb[:, kc, :],
                                start=(kc == 0), stop=(kc == KC - 1))
                    for k in range(2):
                        nc.vector.tensor_copy(
                            out=Vs[t][:, 2 * tp + k, :, 0:D],
                            in_=pv[:, k * TCH:k * TCH + DG]
                            .rearrange("p (h d) -> p h d", h=HG))

            # ---- Q projection emitter (fqpool [128,1024]: mt chains paired) --
            def q_proj_ops(t):
                ops = []
                pq_box = []

                def alloc():
                    pq_box.append(fqpool.tile([128, 2 * TCH], F32,
                                              name="fq", tag="fq"))
                ops.append(alloc)
                for kc in range(KC):
                    for mt in range(2):
                        def mm(kc=kc, mt=mt):
                            nc.tensor.matmul(
                                pq_box[0][:, mt * TCH:(mt + 1) * TCH],
                                wq_sb[:, kc, mt * 128:(mt + 1) * 128],
                                xcs[t][:, kc, :],
                                start=(kc == 0), stop=(kc == KC - 1))
                        ops.append(mm)

                def cp():
                    nc.vector.tensor_copy(
                        out=QTs[t],
                        in_=pq_box[0].rearrange("p (m s) -> p m s", m=2))
                ops.append(cp)
                return ops

            # ---- fc_out emitter for one 512-token chunk (4 x 128 tokens) ----
            def fc_ops(qc):
                ops = []
                for tt in range(TCH // 128):
                    tok = qc * TCH + tt * 128
                    pf_box = []

                    def alloc():
                        pf_box.append(fqpool.tile([128, 1024], F32,
                                                  name="fq", tag="fq"))
                    ops.append(alloc)
                    for hm in range(2):
                        for nch in range(2):
                            def mm(hm=hm, nch=nch, tok=tok):
                                nc.tensor.matmul(
                                    pf_box[0][:, nch * 512:(nch + 1) * 512],
                                    OT2[:, hm, tok:tok + 128],
                                    wo_sb[:, hm, nch * 512:(nch + 1) * 512],
                                    start=(hm == 0), stop=(hm == 1))
                            ops.append(mm)

                    def fin(tok=tok):
                        ys = ypool.tile([128, 1024], MD, name="ys", tag="ys")
                        nc.vector.tensor_copy(out=ys, in_=pf_box[0])
                        nc.sync.dma_start(out=y[tok:tok + 128, :], in_=ys)
                    ops.append(fin)
                return ops

            # Q for the first two q-chunks up front
            for op in q_proj_ops(0) + q_proj_ops(1):
                op()

            OT2 = otpool.tile([128, 2, SEQ], MD)

            # feeder work per qc block (runs inside that block's m-loops)
            feeders = {
                0: q_proj_ops(2),
                1: q_proj_ops(3) + fc_ops(0),
                2: fc_ops(1),
                3: fc_ops(2),
            }

            # ---- fused attention + fed Qproj/fc_out ----
            for qc in range(NT):
                qs = slice(qc * TCH, (qc + 1) * TCH)
                pending = feeders.get(qc, [])
                n_iters_left = 2 * 16

                for hm in range(2):
                    po = popool.tile([D + 1, 2 * TCH], F32, name="po", tag="po")
                    es_prev = None
                    for m in range(16):
                        ps = pspool.tile([128, 2 * TCH], F32,
                                         name="ps", tag="ps")
                        for j in range(2):
                            nc.tensor.matmul(
                                ps[:, j * TCH:(j + 1) * TCH],
                                KTs[m // 4][j * D:(j + 1) * D, hm,
                                            (m % 4) * 128:(m % 4 + 1) * 128],
                                QTs[qc][j * D:(j + 1) * D, hm, :],
                                start=True, stop=True)
                        e = epool.tile([128, 2 * TCH], MD, name="e", tag="e")
                        nc.scalar.activation(
                            out=e, in_=ps,
                            func=mybir.ActivationFunctionType.Exp,
                            scale=1.0 / np.sqrt(D))
                        if es_prev is not None:
                            for j in range(2):
                                nc.tensor.matmul(
                                    po[:, j * TCH:(j + 1) * TCH],
                                    Vs[(m - 1) // 4][:, (m - 1) % 4,
                                                     2 * hm + j, :],
                                    es_prev[:, j * TCH:(j + 1) * TCH],
                                    start=(m == 1), stop=False)
                        es_prev = e
                        # feed pending Qproj/fc work into PE slack
                        n_pop = -(-len(pending) // n_iters_left) \
                            if n_iters_left > 0 else len(pending)
                        for _ in range(min(n_pop, len(pending))):
                            pending.pop(0)()
                        n_iters_left -= 1
                    for j in range(2):
                        nc.tensor.matmul(
                            po[:, j * TCH:(j + 1) * TCH],
                            Vs[3][:, 3, 2 * hm + j, :],
                            es_prev[:, j * TCH:(j + 1) * TCH],
                            start=False, stop=True)

                    # free po fast: one f32 copy, then normalize off-line
                    sc = spool.tile([D + 1, 2 * TCH], F32, name="sc", tag="sc")
                    nc.vector.tensor_copy(out=sc, in_=po)
                    den = denpool.tile([1, 2 * TCH], F32, name="dn", tag="dn")
                    nc.vector.tensor_copy(out=den, in_=sc[D:D + 1, :])
                    rb = rbpool.tile([D, 2 * TCH], F32, name="rb", tag="rb")
                    nc.gpsimd.partition_broadcast(rb, den)
                    nc.vector.reciprocal(out=rb, in_=rb)
                    st = stpool.tile([D, 2 * TCH], MD, name="st", tag="st")
                    nc.vector.tensor_mul(st, sc[0:D, :], rb)
                    for j in range(2):
                        nc.gpsimd.dma_start(
                            out=OT2[j * D:(j + 1) * D, hm, qs],
                            in_=st[:, j * TCH:(j + 1) * TCH])

                while pending:
                    pending.pop(0)()

            # ---- drain: fc_out for the last q-chunk ----
            for op in fc_ops(3):
                op()

    nc.compile()
    return nc


def shard_inputs(x, Wv, Wk, Wq, Wo):
    """Build the 8 per-core input maps."""
    import ml_dtypes
    wire = ml_dtypes.bfloat16
    in_maps = []
    for c in range(NCORES):
        n, g = divmod(c, GROUPS)
        cols = slice(g * DG, (g + 1) * DG)
        in_maps.append({
            "xT": np.ascontiguousarray(
                np.asarray(x[n], np.float32).T).astype(wire),
            "wq": np.ascontiguousarray(
                np.asarray(Wq, np.float32)[:, cols]).astype(wire),
            "wk": np.ascontiguousarray(
                np.asarray(Wk, np.float32)[:, cols]).astype(wire),
            "wv": np.ascontiguousarray(
                np.asarray(Wv, np.float32)[:, cols]).astype(wire),
            "wo": np.ascontiguousarray(
                np.asarray(Wo, np.float32)[cols, :]).astype(wire),
        })
    return in_maps


def kernel(x, Wv, Wk, Wq, Wo, bo):
    global LAST_RESULTS
    x = np.asarray(x, np.float32)
    in_maps = shard_inputs(x, Wv, Wk, Wq, Wo)

    if "nc" not in _CACHED_NC:
        _CACHED_NC["nc"] = build_nc()
    nc = _CACHED_NC["nc"]

    trace = os.environ.get("MHA_TRACE", "0") == "1"
    res = bass_utils.run_bass_kernel_spmd(
        nc, in_maps, core_ids=list(range(NCORES)), trace=trace)
    LAST_RESULTS = res

    bo = np.asarray(bo, np.float32)
    out = np.empty((NB, SEQ, EMBED), np.float32)
    for n in range(NB):
        acc = res.results[n * GROUPS]["y"].astype(np.float32)
        for g in range(1, GROUPS):
            acc = acc + res.results[n * GROUPS + g]["y"].astype(np.float32)
        out[n] = acc + bo[None, :]
    return out


# revision 6
# speedup vs baseline: 1.2461x; 1.0890x over previous
"""Multi-head self-attention Trainium2 kernel (8-core SPMD, full IO).

Problem: x:(2,2048,1024) f32; Wq/Wk/Wv/Wo:(1024,1024); bo:(1024,)
  out = softmax((xWq)(xWk)^T / 8) (xWv) reshaped @ Wo + bo

Sharding: data parallel on batch N=2 x tensor parallel on 16 heads in
4 groups of 4 heads.  Core c handles batch c//4, heads [4*(c%4), 4*(c%4)+4).
Each core computes a partial fc_out product (2048,1024) in bf16; the host
sums the 4 head-group partials per batch (f32) and adds the bias.

v3 schedule (ACT-engine exp stream is the critical path; keep it fed):
  - K/V chunk 0 + Q chunk 0 projected, then attention starts immediately;
    K/V chunks 1-3 are emitted between the m-blocks that first need them.
  - qc-outer attention: per 512-token q-chunk and head pair hm, a
    software-pipelined m-loop: scores S^T (PE, row-grouped j pair), exp
    (ACT, one [128,1024] instr for both heads), A@V lagging two
    iterations so the PE never waits on ACT.
  - remaining Q projections and fc_out matmuls feed into PE slack, max 2
    per iteration; fc_out is gated until iter 6 of its block so it never
    stalls the queue on the previous block's O^T staging DMAs.
  - softmax denominator from a ones-column of V; normalize on-chip:
    scratch copy (frees psum fast), gpsimd partition_broadcast of the
    denominator row, reciprocal, fused multiply into bf16 O^T staging.
"""

import os

import numpy as np

import concourse.bass as bass
import concourse.tile as tile
from concourse import bacc, mybir
from concourse import bass_utils

F32 = mybir.dt.float32
BF16 = mybir.dt.bfloat16

EMBED = 1024
SEQ = 2048
NB = 2
HEADS = 16
D = 64
NCORES = 8
GROUPS = 4
HG = HEADS // GROUPS  # 4 heads per core
DG = HG * D  # 256 dims per core
KC = EMBED // 128  # 8 contraction chunks
TCH = 512  # token chunk
NT = SEQ // TCH  # 4 chunks

_MM_DTYPE_NAME = "bfloat16"
MD = BF16

LAST_RESULTS = None
_CACHED_NC = {}


def build_nc():
    nc = bacc.Bacc("TRN2", target_bir_lowering=False, debug=False,
                   num_devices=NCORES)

    xT = nc.dram_tensor("xT", (EMBED, SEQ), MD, kind="ExternalInput").ap()
    wq = nc.dram_tensor("wq", (EMBED, DG), MD, kind="ExternalInput").ap()
    wk = nc.dram_tensor("wk", (EMBED, DG), MD, kind="ExternalInput").ap()
    wv = nc.dram_tensor("wv", (EMBED, DG), MD, kind="ExternalInput").ap()
    wo = nc.dram_tensor("wo", (DG, EMBED), MD, kind="ExternalInput").ap()
    y = nc.dram_tensor("y", (SEQ, EMBED), MD, kind="ExternalOutput").ap()

    with tile.TileContext(nc) as tc:
        with (
            tc.tile_pool(name="weights", bufs=1) as wpool,
            tc.tile_pool(name="qk", bufs=1) as qkpool,
            tc.tile_pool(name="vpool", bufs=1) as vpool,
            tc.tile_pool(name="otpool", bufs=1) as otpool,
            tc.tile_pool(name="xchunk", bufs=4) as xpool,
            tc.tile_pool(name="epool", bufs=4) as epool,
            tc.tile_pool(name="scratch", bufs=1) as spool,
            tc.tile_pool(name="stage", bufs=1) as stpool,
            tc.tile_pool(name="rbc", bufs=1) as rbpool,
            tc.tile_pool(name="denr", bufs=1) as drpool,
            tc.tile_pool(name="ystage", bufs=2) as ypool,
            tc.tile_pool(name="psum", bufs=2, space="PSUM") as pspool,
            tc.tile_pool(name="psum_o", bufs=1, space="PSUM") as popool,
            tc.tile_pool(name="psum_fcq", bufs=1, space="PSUM") as fqpool,
        ):
            # ---- weights + x chunks ----
            wk_sb = wpool.tile([128, KC, DG], MD)
            wv_sb = wpool.tile([128, KC, DG], MD)
            wq_sb = wpool.tile([128, KC, DG], MD)
            wo_sb = wpool.tile([128, DG // 128, EMBED], MD)
            nc.sync.dma_start(out=wk_sb, in_=wk.rearrange("(c p) n -> p c n", p=128))
            nc.sync.dma_start(out=wv_sb, in_=wv.rearrange("(c p) n -> p c n", p=128))

            xTr = xT.rearrange("(c p) s -> p c s", p=128)
            xcs = []
            for t in range(NT):
                xc = xpool.tile([128, KC, TCH], MD, name=f"xc{t}", tag=f"xc{t}")
                nc.sync.dma_start(out=xc, in_=xTr[:, :, t * TCH:(t + 1) * TCH])
                xcs.append(xc)
                if t == 0:
                    nc.sync.dma_start(
                        out=wq_sb, in_=wq.rearrange("(c p) n -> p c n", p=128))
            nc.sync.dma_start(out=wo_sb, in_=wo.rearrange("(c p) n -> p c n", p=128))

            QTs = [qkpool.tile([128, 2, TCH], MD, name=f"qt{t}", tag=f"qt{t}")
                   for t in range(NT)]
            KTs = [qkpool.tile([128, 2, TCH], MD, name=f"kt{t}", tag=f"kt{t}")
                   for t in range(NT)]
            Vs = [vpool.tile([128, 4, HG, D + 1], MD, name=f"v{t}", tag=f"v{t}")
                  for t in range(NT)]
            for t in range(NT):
                nc.vector.memset(Vs[t][:, :, :, D:D + 1], 1.0)

            # ---- emitters ----
            def kv_proj(t):
                xc = xcs[t]
                pk = pspool.tile([128, 2 * TCH], F32, name="ps", tag="ps")
                for kc in range(KC):
                    for mt in range(2):
                        nc.tensor.matmul(
                            pk[:, mt * TCH:(mt + 1) * TCH],
                            wk_sb[:, kc, mt * 128:(mt + 1) * 128],
                            xc[:, kc, :],
                            start=(kc == 0), stop=(kc == KC - 1))
                nc.vector.tensor_copy(
                    out=KTs[t], in_=pk.rearrange("p (m s) -> p m s", m=2))
                for tp in range(2):
                    pv = pspool.tile([128, 2 * TCH], F32, name="ps", tag="ps")
                    for kc in range(KC):
                        for k in range(2):
                            ti = 2 * tp + k
                            nc.tensor.matmul(
                                pv[:, k * TCH:k * TCH + DG],
                                xc[:, kc, ti * 128:(ti + 1) * 128],
                                wv_sb[:, kc, :],
                                start=(kc == 0), stop=(kc == KC - 1))
                    for k in range(2):
                        nc.vector.tensor_copy(
                            out=Vs[t][:, 2 * tp + k, :, 0:D],
                            in_=pv[:, k * TCH:k * TCH + DG]
                            .rearrange("p (h d) -> p h d", h=HG))

            def q_proj_ops(t):
                """Feeder items ('q', op) for projecting Q chunk t."""
                ops = []
                pq_box = []

                def alloc():
                    pq_box.append(fqpool.tile([128, 2 * TCH], F32,
                                              name="fq", tag="fq"))
                ops.append(('q', alloc))
                for kc in range(KC):
                    for mt in range(2):
                        def mm(kc=kc, mt=mt):
                            nc.tensor.matmul(
                                pq_box[0][:, mt * TCH:(mt + 1) * TCH],
                                wq_sb[:, kc, mt * 128:(mt + 1) * 128],
                                xcs[t][:, kc, :],
                                start=(kc == 0), stop=(kc == KC - 1))
                        ops.append(('q', mm))

                def cp():
                    nc.vector.tensor_copy(
                        out=QTs[t],
                        in_=pq_box[0].rearrange("p (m s) -> p m s", m=2))
                ops.append(('q', cp))
                return ops

            def fc_ops(qc):
                """Feeder items ('fc', op) for fc_out of q-chunk qc."""
                ops = []
                for tt in range(TCH // 128):
                    tok = qc * TCH + tt * 128
                    pf_box = []

                    def alloc():
                        pf_box.append(fqpool.tile([128, 1024], F32,
                                                  name="fq", tag="fq"))
                    ops.append(('fc', alloc))
                    for hm in range(2):
                        for nch in range(2):
                            def mm(hm=hm, nch=nch, tok=tok):
                                nc.tensor.matmul(
                                    pf_box[0][:, nch * 512:(nch + 1) * 512],
                                    OT2[:, hm, tok:tok + 128],
                                    wo_sb[:, hm, nch * 512:(nch + 1) * 512],
                                    start=(hm == 0), stop=(hm == 1))
                            ops.append(('fc', mm))

                    def fin(tok=tok):
                        ys = ypool.tile([128, 1024], MD, name="ys", tag="ys")
                        nc.vector.tensor_copy(out=ys, in_=pf_box[0])
                        nc.sync.dma_start(out=y[tok:tok + 128, :], in_=ys)
                    ops.append(('fc', fin))
                return ops

            OT2 = otpool.tile([128, 2, SEQ], MD)

            # ---- prologue: first K/V chunk + first Q chunk ----
            kv_proj(0)
            for _, op in q_proj_ops(0):
                op()

            feeders = {
                0: q_proj_ops(1),
                1: q_proj_ops(2) + fc_ops(0),
                2: q_proj_ops(3) + fc_ops(1),
                3: fc_ops(2),
            }

            # ---- fused attention ----
            for qc in range(NT):
                qs = slice(qc * TCH, (qc + 1) * TCH)
                pending = feeders.get(qc, [])
                n_iters_left = 2 * 16
                it = 0

                for hm in range(2):
                    po = popool.tile([D + 1, 2 * TCH], F32, name="po", tag="po")
                    es = [None, None]  # lag-2 pipeline: es[m-2], es[m-1]
                    for m in range(16):
                        # late K/V chunks, right before first use (qc 0 only)
                        if qc == 0 and hm == 0 and m in (4, 8, 12):
                            kv_proj(m // 4)
                        ps = pspool.tile([128, 2 * TCH], F32,
                                         name="ps", tag="ps")
                        for j in range(2):
                            nc.tensor.matmul(
                                ps[:, j * TCH:(j + 1) * TCH],
                                KTs[m // 4][j * D:(j + 1) * D, hm,
                                            (m % 4) * 128:(m % 4 + 1) * 128],
                                QTs[qc][j * D:(j + 1) * D, hm, :],
                                start=True, stop=True)
                        e = epool.tile([128, 2 * TCH], MD, name="e", tag="e")
                        nc.scalar.activation(
                            out=e, in_=ps,
                            func=mybir.ActivationFunctionType.Exp,
                            scale=1.0 / np.sqrt(D))
                        if es[0] is not None:
                            for j in range(2):
                                nc.tensor.matmul(
                                    po[:, j * TCH:(j + 1) * TCH],
                                    Vs[(m - 2) // 4][:, (m - 2) % 4,
                                                     2 * hm + j, :],
                                    es[0][:, j * TCH:(j + 1) * TCH],
                                    start=(m == 2), stop=False)
                        es = [es[1], e]
                        # feed pending work into PE slack (<=2 per iter;
                        # fc gated to iter>=6 of this qc block)
                        n_pop = min(2, -(-len(pending) // n_iters_left)
                                    if n_iters_left > 0 else len(pending))
                        for _ in range(n_pop):
                            if not pending:
                                break
                            tag, op = pending[0]
                            if tag == 'fc' and it < 6:
                                break
                            pending.pop(0)
                            op()
                        n_iters_left -= 1
                        it += 1
                    # drain the two lagged AV pairs
                    for mm_, e_ in ((14, es[0]), (15, es[1])):
                        for j in range(2):
                            nc.tensor.matmul(
                                po[:, j * TCH:(j + 1) * TCH],
                                Vs[3][:, 3 if mm_ == 15 else 2,
                                      2 * hm + j, :],
                                e_[:, j * TCH:(j + 1) * TCH],
                                start=False, stop=(mm_ == 15))

                    # normalize: scratch copy frees psum, then
                    # broadcast(den row) -> reciprocal -> fused mul
                    sc = spool.tile([D + 1, 2 * TCH], F32, name="sc", tag="sc")
                    nc.vector.tensor_copy(out=sc, in_=po)
                    dr = drpool.tile([1, 2 * TCH], F32, name="dr", tag="dr")
                    nc.vector.reciprocal(out=dr, in_=sc[D:D + 1, :])
                    rb = rbpool.tile([D, 2 * TCH], F32, name="rb", tag="rb")
                    nc.gpsimd.partition_broadcast(rb, dr)
                    st = stpool.tile([D, 2 * TCH], MD, name="st", tag="st")
                    nc.vector.tensor_mul(st, sc[0:D, :], rb)
                    for j in range(2):
                        nc.gpsimd.dma_start(
                            out=OT2[j * D:(j + 1) * D, hm, qs],
                            in_=st[:, j * TCH:(j + 1) * TCH])

                while pending:
                    pending.pop(0)[1]()

            for _, op in fc_ops(3):
                op()

    nc.compile()
    return nc


def shard_inputs(x, Wv, Wk, Wq, Wo):
    import ml_dtypes
    wire = ml_dtypes.bfloat16
    in_maps = []
    for c in range(NCORES):
        n, g = divmod(c, GROUPS)
        cols = slice(g * DG, (g + 1) * DG)
        in_maps.append({
            "xT": np.ascontiguousarray(
                np.asarray(x[n], np.float32).T).astype(wire),
            "wq": np.ascontiguousarray(
                np.asarray(Wq, np.float32)[:, cols]).astype(wire),
            "wk": np.ascontiguousarray(
                np.asarray(Wk, np.float32)[:, cols]).astype(wire),
            "wv": np.ascontiguousarray(
                np.asarray(Wv, np.float32)[:, cols]).astype(wire),
            "wo": np.ascontiguousarray(
                np.asarray(Wo, np.float32)[cols, :]).astype(wire),
        })
    return in_maps


def kernel(x, Wv, Wk, Wq, Wo, bo):
    global LAST_RESULTS
    x = np.asarray(x, np.float32)
    in_maps = shard_inputs(x, Wv, Wk, Wq, Wo)

    if "nc" not in _CACHED_NC:
        _CACHED_NC["nc"] = build_nc()
    nc = _CACHED_NC["nc"]

    trace = os.environ.get("MHA_TRACE", "0") == "1"
    res = bass_utils.run_bass_kernel_spmd(
        nc, in_maps, core_ids=list(range(NCORES)), trace=trace)
    LAST_RESULTS = res

    bo = np.asarray(bo, np.float32)
    out = np.empty((NB, SEQ, EMBED), np.float32)
    for n in range(NB):
        acc = res.results[n * GROUPS]["y"].astype(np.float32)
        for g in range(1, GROUPS):
            acc = acc + res.results[n * GROUPS + g]["y"].astype(np.float32)
        out[n] = acc + bo[None, :]
    return out


# revision 22
# speedup vs baseline: 1.2473x; 1.0009x over previous
"""Multi-head self-attention Trainium2 kernel (8-core SPMD, full IO).

Problem: x:(2,2048,1024) f32; Wq/Wk/Wv/Wo:(1024,1024); bo:(1024,)
  out = softmax((xWq)(xWk)^T / 8) (xWv) reshaped @ Wo + bo

Sharding: data parallel on batch N=2 x tensor parallel on 16 heads in
4 groups of 4 heads.  Core c handles batch c//4, heads [4*(c%4), 4*(c%4)+4).
Each core computes a partial fc_out product (2048,1024) in bf16; the host
sums the 4 head-group partials per batch (f32) and adds the bias.

v3 schedule (ACT-engine exp stream is the critical path; keep it fed):
  - K/V chunk 0 + Q chunk 0 projected, then attention starts immediately;
    K/V chunks 1-3 are emitted between the m-blocks that first need them.
  - qc-outer attention: per 512-token q-chunk and head pair hm, a
    software-pipelined m-loop: scores S^T (PE, row-grouped j pair), exp
    (ACT, one [128,1024] instr for both heads), A@V lagging two
    iterations so the PE never waits on ACT.
  - remaining Q projections and fc_out matmuls feed into PE slack, max 2
    per iteration; fc_out is gated until iter 6 of its block so it never
    stalls the queue on the previous block's O^T staging DMAs.
  - softmax denominator from a ones-column of V; normalize on-chip:
    scratch copy (frees psum fast), gpsimd partition_broadcast of the
    denominator row, reciprocal, fused multiply into bf16 O^T staging.
"""

import os

import numpy as np

import concourse.bass as bass
import concourse.tile as tile
from concourse import bacc, mybir
from concourse import bass_utils

F32 = mybir.dt.float32
BF16 = mybir.dt.bfloat16

EMBED = 1024
SEQ = 2048
NB = 2
HEADS = 16
D = 64
NCORES = 8
GROUPS = 4
HG = HEADS // GROUPS  # 4 heads per core
DG = HG * D  # 256 dims per core
KC = EMBED // 128  # 8 contraction chunks
TCH = 512  # token chunk
NT = SEQ // TCH  # 4 chunks

_MM_DTYPE_NAME = "bfloat16"
MD = BF16

LAST_RESULTS = None
_CACHED_NC = {}


def build_nc():
    nc = bacc.Bacc("TRN2", target_bir_lowering=False, debug=False,
                   num_devices=NCORES)

    xT = nc.dram_tensor("xT", (EMBED, SEQ), MD, kind="ExternalInput").ap()
    wq = nc.dram_tensor("wq", (EMBED, DG), MD, kind="ExternalInput").ap()
    wk = nc.dram_tensor("wk", (EMBED, DG), MD, kind="ExternalInput").ap()
    wv = nc.dram_tensor("wv", (EMBED, DG), MD, kind="ExternalInput").ap()
    wo = nc.dram_tensor("wo", (DG, EMBED), MD, kind="ExternalInput").ap()
    y = nc.dram_tensor("y", (SEQ, EMBED), MD, kind="ExternalOutput").ap()

    with tile.TileContext(nc) as tc:
        with (
            tc.tile_pool(name="weights", bufs=1) as wpool,
            tc.tile_pool(name="qk", bufs=1) as qkpool,
            tc.tile_pool(name="vpool", bufs=1) as vpool,
            tc.tile_pool(name="otpool", bufs=1) as otpool,
            tc.tile_pool(name="xchunk", bufs=4) as xpool,
            tc.tile_pool(name="epool", bufs=4) as epool,
            tc.tile_pool(name="scratch", bufs=1) as spool,
            tc.tile_pool(name="stage", bufs=1) as stpool,
            tc.tile_pool(name="rbc", bufs=1) as rbpool,
            tc.tile_pool(name="denr", bufs=1) as drpool,
            tc.tile_pool(name="ystage", bufs=2) as ypool,
            tc.tile_pool(name="psum", bufs=2, space="PSUM") as pspool,
            tc.tile_pool(name="psum_o", bufs=1, space="PSUM") as popool,
            tc.tile_pool(name="psum_fcq", bufs=1, space="PSUM") as fqpool,
        ):
            # ---- weights + x chunks ----
            wk_sb = wpool.tile([128, KC, DG], MD)
            wv_sb = wpool.tile([128, KC, DG], MD)
            wq_sb = wpool.tile([128, KC, DG], MD)
            wo_sb = wpool.tile([128, DG // 128, EMBED], MD)
            # spread the startup DMAs over several queues so the first
            # matmul's inputs (wk + x chunk 0) land as early as possible
            nc.sync.dma_start(out=wk_sb, in_=wk.rearrange("(c p) n -> p c n", p=128))
            nc.scalar.dma_start(out=wv_sb, in_=wv.rearrange("(c p) n -> p c n", p=128))

            xTr = xT.rearrange("(c p) s -> p c s", p=128)
            xcs = []
            for t in range(NT):
                xc = xpool.tile([128, KC, TCH], MD, name=f"xc{t}", tag=f"xc{t}")
                nc.sync.dma_start(out=xc, in_=xTr[:, :, t * TCH:(t + 1) * TCH])
                xcs.append(xc)
                if t == 0:
                    nc.scalar.dma_start(
                        out=wq_sb, in_=wq.rearrange("(c p) n -> p c n", p=128))
            nc.scalar.dma_start(out=wo_sb, in_=wo.rearrange("(c p) n -> p c n", p=128))

            QTs = [qkpool.tile([128, 2, TCH], MD, name=f"qt{t}", tag=f"qt{t}")
                   for t in range(NT)]
            KTs = [qkpool.tile([128, 2, TCH], MD, name=f"kt{t}", tag=f"kt{t}")
                   for t in range(NT)]
            Vs = [vpool.tile([128, 4, HG, D + 1], MD, name=f"v{t}", tag=f"v{t}")
                  for t in range(NT)]
            for t in range(NT):
                nc.vector.memset(Vs[t][:, :, :, D:D + 1], 1.0)

            # ---- emitters ----
            def kv_proj(t):
                xc = xcs[t]
                pk = pspool.tile([128, 2 * TCH], F32, name="ps", tag="ps")
                for kc in range(KC):
                    for mt in range(2):
                        nc.tensor.matmul(
                            pk[:, mt * TCH:(mt + 1) * TCH],
                            wk_sb[:, kc, mt * 128:(mt + 1) * 128],
                            xc[:, kc, :],
                            start=(kc == 0), stop=(kc == KC - 1))
                nc.vector.tensor_copy(
                    out=KTs[t], in_=pk.rearrange("p (m s) -> p m s", m=2))
                for tp in range(2):
                    pv = pspool.tile([128, 2 * TCH], F32, name="ps", tag="ps")
                    for kc in range(KC):
                        for k in range(2):
                            ti = 2 * tp + k
                            nc.tensor.matmul(
                                pv[:, k * TCH:k * TCH + DG],
                                xc[:, kc, ti * 128:(ti + 1) * 128],
                                wv_sb[:, kc, :],
                                start=(kc == 0), stop=(kc == KC - 1))
                    for k in range(2):
                        nc.vector.tensor_copy(
                            out=Vs[t][:, 2 * tp + k, :, 0:D],
                            in_=pv[:, k * TCH:k * TCH + DG]
                            .rearrange("p (h d) -> p h d", h=HG))

            def q_proj_ops(t):
                """Feeder items ('q', op) for projecting Q chunk t."""
                ops = []
                pq_box = []

                def alloc():
                    pq_box.append(fqpool.tile([128, 2 * TCH], F32,
                                              name="fq", tag="fq"))
                ops.append(('q', alloc))
                for kc in range(KC):
                    for mt in range(2):
                        def mm(kc=kc, mt=mt):
                            nc.tensor.matmul(
                                pq_box[0][:, mt * TCH:(mt + 1) * TCH],
                                wq_sb[:, kc, mt * 128:(mt + 1) * 128],
                                xcs[t][:, kc, :],
                                start=(kc == 0), stop=(kc == KC - 1))
                        ops.append(('q', mm))

                def cp():
                    nc.vector.tensor_copy(
                        out=QTs[t],
                        in_=pq_box[0].rearrange("p (m s) -> p m s", m=2))
                ops.append(('q', cp))
                return ops

            def fc_ops(qc):
                """Feeder items ('fc', op) for fc_out of q-chunk qc."""
                ops = []
                for tt in range(TCH // 128):
                    tok = qc * TCH + tt * 128
                    pf_box = []

                    def alloc():
                        pf_box.append(fqpool.tile([128, 1024], F32,
                                                  name="fq", tag="fq"))
                    ops.append(('fc', alloc))
                    for hm in range(2):
                        for nch in range(2):
                            def mm(hm=hm, nch=nch, tok=tok):
                                nc.tensor.matmul(
                                    pf_box[0][:, nch * 512:(nch + 1) * 512],
                                    OT2[:, hm, tok:tok + 128],
                                    wo_sb[:, hm, nch * 512:(nch + 1) * 512],
                                    start=(hm == 0), stop=(hm == 1))
                            ops.append(('fc', mm))

                    def fin(tok=tok):
                        ys = ypool.tile([128, 1024], MD, name="ys", tag="ys")
                        nc.vector.tensor_copy(out=ys, in_=pf_box[0])
                        nc.sync.dma_start(out=y[tok:tok + 128, :], in_=ys)
                    ops.append(('fc', fin))
                return ops

            OT2 = otpool.tile([128, 2, SEQ], MD)

            # ---- prologue: first K/V chunk + first Q chunk ----
            kv_proj(0)
            for _, op in q_proj_ops(0):
                op()

            feeders = {
                0: q_proj_ops(1),
                1: q_proj_ops(2) + fc_ops(0),
                2: q_proj_ops(3) + fc_ops(1),
                3: fc_ops(2),
            }

            # ---- fused attention ----
            for qc in range(NT):
                qs = slice(qc * TCH, (qc + 1) * TCH)
                pending = feeders.get(qc, [])
                n_iters_left = 2 * 16
                it = 0

                for hm in range(2):
                    po = popool.tile([D + 1, 2 * TCH], F32, name="po", tag="po")
                    es = [None, None]  # lag-2 pipeline: es[m-2], es[m-1]
                    for m in range(16):
                        # late K/V chunks, right before first use (qc 0 only)
                        if qc == 0 and hm == 0 and m in (4, 8, 12):
                            kv_proj(m // 4)
                        ps = pspool.tile([128, 2 * TCH], F32,
                                         name="ps", tag="ps")
                        for j in range(2):
                            nc.tensor.matmul(
                                ps[:, j * TCH:(j + 1) * TCH],
                                KTs[m // 4][j * D:(j + 1) * D, hm,
                                            (m % 4) * 128:(m % 4 + 1) * 128],
                                QTs[qc][j * D:(j + 1) * D, hm, :],
                                start=True, stop=True)
                        e = epool.tile([128, 2 * TCH], MD, name="e", tag="e")
                        nc.scalar.activation(
                            out=e, in_=ps,
                            func=mybir.ActivationFunctionType.Exp,
                            scale=1.0 / np.sqrt(D))
                        if es[0] is not None:
                            for j in range(2):
                                nc.tensor.matmul(
                                    po[:, j * TCH:(j + 1) * TCH],
                                    Vs[(m - 2) // 4][:, (m - 2) % 4,
                                                     2 * hm + j, :],
                                    es[0][:, j * TCH:(j + 1) * TCH],
                                    start=(m == 2), stop=False)
                        es = [es[1], e]
                        # feed pending work into PE slack (<=2 per iter;
                        # fc gated to iter>=6 of this qc block)
                        n_pop = min(2, -(-len(pending) // n_iters_left)
                                    if n_iters_left > 0 else len(pending))
                        for _ in range(n_pop):
                            if not pending:
                                break
                            tag, op = pending[0]
                            if tag == 'fc' and it < 10:
                                break
                            pending.pop(0)
                            op()
                        n_iters_left -= 1
                        it += 1
                    # drain the two lagged AV pairs
                    for mm_, e_ in ((14, es[0]), (15, es[1])):
                        for j in range(2):
                            nc.tensor.matmul(
                                po[:, j * TCH:(j + 1) * TCH],
                                Vs[3][:, 3 if mm_ == 15 else 2,
                                      2 * hm + j, :],
                                e_[:, j * TCH:(j + 1) * TCH],
                                start=False, stop=(mm_ == 15))

                    # normalize: scratch copy frees psum, then
                    # broadcast(den row) -> reciprocal -> fused mul
                    sc = spool.tile([D + 1, 2 * TCH], F32, name="sc", tag="sc")
                    nc.vector.tensor_copy(out=sc, in_=po)
                    dr = drpool.tile([1, 2 * TCH], F32, name="dr", tag="dr")
                    nc.vector.reciprocal(out=dr, in_=sc[D:D + 1, :])
                    rb = rbpool.tile([D, 2 * TCH], F32, name="rb", tag="rb")
                    nc.gpsimd.partition_broadcast(rb, dr)
                    st = stpool.tile([D, 2 * TCH], MD, name="st", tag="st")
                    nc.vector.tensor_mul(st, sc[0:D, :], rb)
                    for j in range(2):
                        nc.gpsimd.dma_start(
                            out=OT2[j * D:(j + 1) * D, hm, qs],
                            in_=st[:, j * TCH:(j + 1) * TCH])

                while pending:
                    pending.pop(0)[1]()

            for _, op in fc_ops(3):
                op()

    nc.compile()
    return nc


def shard_inputs(x, Wv, Wk, Wq, Wo):
    import ml_dtypes
    wire = ml_dtypes.bfloat16
    in_maps = []
    for c in range(NCORES):
        n, g = divmod(c, GROUPS)
        cols = slice(g * DG, (g + 1) * DG)
        in_maps.append({
            "xT": np.ascontiguousarray(
                np.asarray(x[n], np.float32).T).astype(wire),
            "wq": np.ascontiguousarray(
                np.asarray(Wq, np.float32)[:, cols]).astype(wire),
            "wk": np.ascontiguousarray(
                np.asarray(Wk, np.float32)[:, cols]).astype(wire),
            "wv": np.ascontiguousarray(
                np.asarray(Wv, np.float32)[:, cols]).astype(wire),
            "wo": np.ascontiguousarray(
                np.asarray(Wo, np.float32)[cols, :]).astype(wire),
        })
    return in_maps


def kernel(x, Wv, Wk, Wq, Wo, bo):
    global LAST_RESULTS
    x = np.asarray(x, np.float32)
    in_maps = shard_inputs(x, Wv, Wk, Wq, Wo)

    if "nc" not in _CACHED_NC:
        _CACHED_NC["nc"] = build_nc()
    nc = _CACHED_NC["nc"]

    trace = os.environ.get("MHA_TRACE", "0") == "1"
    res = bass_utils.run_bass_kernel_spmd(
        nc, in_maps, core_ids=list(range(NCORES)), trace=trace)
    LAST_RESULTS = res

    bo = np.asarray(bo, np.float32)
    out = np.empty((NB, SEQ, EMBED), np.float32)
    for n in range(NB):
        acc = res.results[n * GROUPS]["y"].astype(np.float32)
        for g in range(1, GROUPS):
            acc = acc + res.results[n * GROUPS + g]["y"].astype(np.float32)
        out[n] = acc + bo[None, :]
    return out


# revision 29
# speedup vs baseline: 1.4912x; 1.1956x over previous
"""Multi-head self-attention Trainium2 kernel (8-core SPMD, full IO).

Problem: x:(2,2048,1024) f32; Wq/Wk/Wv/Wo:(1024,1024); bo:(1024,)
  out = softmax((xWq)(xWk)^T / 8) (xWv) reshaped @ Wo + bo

Sharding: data parallel on batch N=2 x tensor parallel on 16 heads in
4 groups of 4 heads.  Core c handles batch c//4, heads [4*(c%4), 4*(c%4)+4).
Each core computes a partial fc_out product (2048,1024) in bf16; the host
sums the 4 head-group partials per batch (f32) and adds the bias.

v3 schedule (ACT-engine exp stream is the critical path; keep it fed):
  - K/V chunk 0 + Q chunk 0 projected, then attention starts immediately;
    K/V chunks 1-3 are emitted between the m-blocks that first need them.
  - qc-outer attention: per 512-token q-chunk and head pair hm, a
    software-pipelined m-loop: scores S^T (PE, row-grouped j pair), exp
    (ACT, one [128,1024] instr for both heads), A@V lagging two
    iterations so the PE never waits on ACT.
  - remaining Q projections and fc_out matmuls feed into PE slack, max 2
    per iteration; fc_out is gated until iter 6 of its block so it never
    stalls the queue on the previous block's O^T staging DMAs.
  - softmax denominator from a ones-column of V; normalize on-chip:
    scratch copy (frees psum fast), gpsimd partition_broadcast of the
    denominator row, reciprocal, fused multiply into bf16 O^T staging.
"""

import os

import numpy as np

import concourse.bass as bass
import concourse.tile as tile
from concourse import bacc, mybir
from concourse import bass_utils

F32 = mybir.dt.float32
BF16 = mybir.dt.bfloat16

EMBED = 1024
SEQ = 2048
NB = 2
HEADS = 16
D = 64
NCORES = 8
GROUPS = 4
HG = HEADS // GROUPS  # 4 heads per core
DG = HG * D  # 256 dims per core
KC = EMBED // 128  # 8 contraction chunks
TCH = 512  # token chunk
NT = SEQ // TCH  # 4 chunks

_MM_DTYPE_NAME = "bfloat16"
MD = BF16

LAST_RESULTS = None
_CACHED_NC = {}


def build_nc():
    nc = bacc.Bacc("TRN2", target_bir_lowering=False, debug=False,
                   num_devices=NCORES)

    xT = nc.dram_tensor("xT", (EMBED, SEQ), MD, kind="ExternalInput").ap()
    wq = nc.dram_tensor("wq", (EMBED, DG), MD, kind="ExternalInput").ap()
    wk = nc.dram_tensor("wk", (EMBED, DG), MD, kind="ExternalInput").ap()
    wv = nc.dram_tensor("wv", (EMBED, DG), MD, kind="ExternalInput").ap()
    wo = nc.dram_tensor("wo", (DG, EMBED), MD, kind="ExternalInput").ap()
    y = nc.dram_tensor("y", (SEQ, EMBED), MD, kind="ExternalOutput").ap()

    with tile.TileContext(nc) as tc:
        with (
            tc.tile_pool(name="weights", bufs=1) as wpool,
            tc.tile_pool(name="qk", bufs=1) as qkpool,
            tc.tile_pool(name="vpool", bufs=1) as vpool,
            tc.tile_pool(name="otpool", bufs=1) as otpool,
            tc.tile_pool(name="xchunk", bufs=4) as xpool,
            tc.tile_pool(name="epool", bufs=4) as epool,
            tc.tile_pool(name="scratch", bufs=1) as spool,
            tc.tile_pool(name="stage", bufs=1) as stpool,
            tc.tile_pool(name="rbc", bufs=1) as rbpool,
            tc.tile_pool(name="denr", bufs=1) as drpool,
            tc.tile_pool(name="ystage", bufs=2) as ypool,
            tc.tile_pool(name="psum", bufs=2, space="PSUM") as pspool,
            tc.tile_pool(name="psum_o", bufs=1, space="PSUM") as popool,
            tc.tile_pool(name="psum_fcq", bufs=1, space="PSUM") as fqpool,
        ):
            # ---- weights + x chunks ----
            wk_sb = wpool.tile([128, KC, DG], MD)
            wv_sb = wpool.tile([128, KC, DG], MD)
            wq_sb = wpool.tile([128, KC, DG], MD)
            wo_sb = wpool.tile([128, DG // 128, EMBED], MD)
            # spread the startup DMAs over two queues, splitting the
            # first-matmul inputs (wk + x chunk 0) in half across both so
            # the K projection can start as early as possible
            wkr = wk.rearrange("(c p) n -> p c n", p=128)
            nc.sync.dma_start(out=wk_sb[:, 0:KC // 2], in_=wkr[:, 0:KC // 2])
            nc.scalar.dma_start(out=wk_sb[:, KC // 2:], in_=wkr[:, KC // 2:])

            xTr = xT.rearrange("(c p) s -> p c s", p=128)
            xcs = []
            for t in range(NT):
                xc = xpool.tile([128, KC, TCH], MD, name=f"xc{t}", tag=f"xc{t}")
                xv = xTr[:, :, t * TCH:(t + 1) * TCH]
                if t == 0:
                    nc.sync.dma_start(out=xc[:, 0:KC // 2], in_=xv[:, 0:KC // 2])
                    nc.scalar.dma_start(out=xc[:, KC // 2:], in_=xv[:, KC // 2:])
                    nc.scalar.dma_start(
                        out=wv_sb, in_=wv.rearrange("(c p) n -> p c n", p=128))
                    nc.scalar.dma_start(
                        out=wq_sb, in_=wq.rearrange("(c p) n -> p c n", p=128))
                else:
                    nc.sync.dma_start(out=xc, in_=xv)
                xcs.append(xc)
            nc.scalar.dma_start(out=wo_sb, in_=wo.rearrange("(c p) n -> p c n", p=128))

            QTs = [qkpool.tile([128, 2, TCH], MD, name=f"qt{t}", tag=f"qt{t}")
                   for t in range(NT)]
            KTs = [qkpool.tile([128, 2, TCH], MD, name=f"kt{t}", tag=f"kt{t}")
                   for t in range(NT)]
            Vs = [vpool.tile([128, 4, HG, D + 1], MD, name=f"v{t}", tag=f"v{t}")
                  for t in range(NT)]
            for t in range(NT):
                nc.vector.memset(Vs[t][:, :, :, D:D + 1], 1.0)

            # ---- emitters ----
            def kv_proj(t):
                xc = xcs[t]
                pk = pspool.tile([128, 2 * TCH], F32, name="ps", tag="ps")
                for kc in range(KC):
                    for mt in range(2):
                        nc.tensor.matmul(
                            pk[:, mt * TCH:(mt + 1) * TCH],
                            wk_sb[:, kc, mt * 128:(mt + 1) * 128],
                            xc[:, kc, :],
                            start=(kc == 0), stop=(kc == KC - 1))
                nc.vector.tensor_copy(
                    out=KTs[t], in_=pk.rearrange("p (m s) -> p m s", m=2))
                for tp in range(2):
                    pv = pspool.tile([128, 2 * TCH], F32, name="ps", tag="ps")
                    for kc in range(KC):
                        for k in range(2):
                            ti = 2 * tp + k
                            nc.tensor.matmul(
                                pv[:, k * TCH:k * TCH + DG],
                                xc[:, kc, ti * 128:(ti + 1) * 128],
                                wv_sb[:, kc, :],
                                start=(kc == 0), stop=(kc == KC - 1))
                    for k in range(2):
                        nc.vector.tensor_copy(
                            out=Vs[t][:, 2 * tp + k, :, 0:D],
                            in_=pv[:, k * TCH:k * TCH + DG]
                            .rearrange("p (h d) -> p h d", h=HG))

            def q_proj_ops(t):
                """Feeder items ('q', op) for projecting Q chunk t."""
                ops = []
                pq_box = []

                def alloc():
                    pq_box.append(fqpool.tile([128, 2 * TCH], F32,
                                              name="fq", tag="fq"))
                ops.append(('q', alloc))
                for kc in range(KC):
                    for mt in range(2):
                        def mm(kc=kc, mt=mt):
                            nc.tensor.matmul(
                                pq_box[0][:, mt * TCH:(mt + 1) * TCH],
                                wq_sb[:, kc, mt * 128:(mt + 1) * 128],
                                xcs[t][:, kc, :],
                                start=(kc == 0), stop=(kc == KC - 1))
                        ops.append(('q', mm))

                def cp():
                    nc.vector.tensor_copy(
                        out=QTs[t],
                        in_=pq_box[0].rearrange("p (m s) -> p m s", m=2))
                ops.append(('q', cp))
                return ops

            def fc_ops(qc, pool=None, tag="fq"):
                """Feeder items ('fc', op) for fc_out of q-chunk qc."""
                ops = []
                for tt in range(TCH // 128):
                    tok = qc * TCH + tt * 128
                    pf_box = []

                    def alloc(pool=pool, tag=tag):
                        pf_box.append((pool or fqpool).tile(
                            [128, 1024], F32, name=tag, tag=tag))
                    ops.append(('fc', alloc))
                    for hm in range(2):
                        for nch in range(2):
                            def mm(hm=hm, nch=nch, tok=tok):
                                nc.tensor.matmul(
                                    pf_box[0][:, nch * 512:(nch + 1) * 512],
                                    OT2[:, hm, tok:tok + 128],
                                    wo_sb[:, hm, nch * 512:(nch + 1) * 512],
                                    start=(hm == 0), stop=(hm == 1))
                            ops.append(('fc', mm))

                    def fin(tok=tok):
                        ys = ypool.tile([128, 1024], MD, name="ys", tag="ys")
                        nc.vector.tensor_copy(out=ys, in_=pf_box[0])
                        nc.sync.dma_start(out=y[tok:tok + 128, :], in_=ys)
                    ops.append(('fc', fin))
                return ops

            OT2 = otpool.tile([128, 2, SEQ], MD)

            # ---- prologue: first K/V chunk + first Q chunk ----
            kv_proj(0)
            for _, op in q_proj_ops(0):
                op()

            feeders = {
                0: q_proj_ops(1),
                1: q_proj_ops(2) + fc_ops(0),
                2: q_proj_ops(3) + fc_ops(1),
                3: fc_ops(2),
            }

            # ---- fused attention ----
            for qc in range(NT):
                qs = slice(qc * TCH, (qc + 1) * TCH)
                pending = feeders.get(qc, [])
                n_iters_left = 2 * 16
                it = 0

                for hm in range(2):
                    po = popool.tile([D + 1, 2 * TCH], F32, name="po", tag="po")
                    es = [None, None]  # lag-2 pipeline: es[m-2], es[m-1]
                    for m in range(16):
                        # late K/V chunks, right before first use (qc 0 only)
                        if qc == 0 and hm == 0 and m in (4, 8, 12):
                            kv_proj(m // 4)
                        ps = pspool.tile([128, 2 * TCH], F32,
                                         name="ps", tag="ps")
                        for j in range(2):
                            nc.tensor.matmul(
                                ps[:, j * TCH:(j + 1) * TCH],
                                KTs[m // 4][j * D:(j + 1) * D, hm,
                                            (m % 4) * 128:(m % 4 + 1) * 128],
                                QTs[qc][j * D:(j + 1) * D, hm, :],
                                start=True, stop=True)
                        e = epool.tile([128, 2 * TCH], MD, name="e", tag="e")
                        nc.scalar.activation(
                            out=e, in_=ps,
                            func=mybir.ActivationFunctionType.Exp,
                            scale=1.0 / np.sqrt(D))
                        if es[0] is not None:
                            for j in range(2):
                                nc.tensor.matmul(
                                    po[:, j * TCH:(j + 1) * TCH],
                                    Vs[(m - 2) // 4][:, (m - 2) % 4,
                                                     2 * hm + j, :],
                                    es[0][:, j * TCH:(j + 1) * TCH],
                                    start=(m == 2), stop=False)
                        es = [es[1], e]
                        # feed pending work into PE slack (<=2 per iter;
                        # fc gated to iter>=6 of this qc block)
                        n_pop = min(2, -(-len(pending) // n_iters_left)
                                    if n_iters_left > 0 else len(pending))
                        for _ in range(n_pop):
                            if not pending:
                                break
                            tag, op = pending[0]
                            if tag == 'fc' and it < 10:
                                break
                            pending.pop(0)
                            op()
                        n_iters_left -= 1
                        it += 1
                    # drain the two lagged AV pairs
                    for mm_, e_ in ((14, es[0]), (15, es[1])):
                        for j in range(2):
                            nc.tensor.matmul(
                                po[:, j * TCH:(j + 1) * TCH],
                                Vs[3][:, 3 if mm_ == 15 else 2,
                                      2 * hm + j, :],
                                e_[:, j * TCH:(j + 1) * TCH],
                                start=False, stop=(mm_ == 15))

                    # normalize: scratch copy frees psum, then
                    # broadcast(den row) -> reciprocal -> fused mul
                    dn = drpool.tile([1, 2 * TCH], F32, name="dn", tag="dn")
                    nc.vector.tensor_copy(out=dn, in_=po[D:D + 1, :])
                    sc = spool.tile([D + 1, 2 * TCH], F32, name="sc", tag="sc")
                    nc.vector.tensor_copy(out=sc, in_=po)
                    dr = drpool.tile([1, 2 * TCH], F32, name="dr", tag="dr")
                    nc.vector.reciprocal_approx_fast(out=dr, in_=dn)
                    rb = rbpool.tile([D, 2 * TCH], F32, name="rb", tag="rb")
                    nc.gpsimd.partition_broadcast(rb, dr)
                    st = stpool.tile([D, 2 * TCH], MD, name="st", tag="st")
                    nc.vector.tensor_mul(st, sc[0:D, :], rb)
                    for j in range(2):
                        nc.gpsimd.dma_start(
                            out=OT2[j * D:(j + 1) * D, hm, qs],
                            in_=st[:, j * TCH:(j + 1) * TCH])

                while pending:
                    pending.pop(0)[1]()

            # drain fc for the last q-chunk from the (now idle) attention
            # psum pool so consecutive groups pipeline instead of
            # serializing on a single buffer
            for _, op in fc_ops(3, pool=pspool, tag="ps"):
                op()

    nc.compile()
    return nc


def shard_inputs(x, Wv, Wk, Wq, Wo):
    import ml_dtypes
    wire = ml_dtypes.bfloat16
    in_maps = []
    for c in range(NCORES):
        n, g = divmod(c, GROUPS)
        cols = slice(g * DG, (g + 1) * DG)
        in_maps.append({
            "xT": np.ascontiguousarray(
                np.asarray(x[n], np.float32).T).astype(wire),
            "wq": np.ascontiguousarray(
                np.asarray(Wq, np.float32)[:, cols]).astype(wire),
            "wk": np.ascontiguousarray(
                np.asarray(Wk, np.float32)[:, cols]).astype(wire),
            "wv": np.ascontiguousarray(
                np.asarray(Wv, np.float32)[:, cols]).astype(wire),
            "wo": np.ascontiguousarray(
                np.asarray(Wo, np.float32)[cols, :]).astype(wire),
        })
    return in_maps


def kernel(x, Wv, Wk, Wq, Wo, bo):
    global LAST_RESULTS
    x = np.asarray(x, np.float32)
    in_maps = shard_inputs(x, Wv, Wk, Wq, Wo)

    if "nc" not in _CACHED_NC:
        _CACHED_NC["nc"] = build_nc()
    nc = _CACHED_NC["nc"]

    trace = os.environ.get("MHA_TRACE", "0") == "1"
    res = bass_utils.run_bass_kernel_spmd(
        nc, in_maps, core_ids=list(range(NCORES)), trace=trace)
    LAST_RESULTS = res

    bo = np.asarray(bo, np.float32)
    out = np.empty((NB, SEQ, EMBED), np.float32)
    for n in range(NB):
        acc = res.results[n * GROUPS]["y"].astype(np.float32)
        for g in range(1, GROUPS):
            acc = acc + res.results[n * GROUPS + g]["y"].astype(np.float32)
        out[n] = acc + bo[None, :]
    return out
